# revision 34
# baseline (speedup 1.0000x reference)
"""LIF spike kernel for Trainium2 (Bass/Tile), data-parallel over batch on 8 cores.

Host layout per core: x_core [C=128, T=8, F=B_loc*HW=4096] f32, split into
column GROUPS, each an independent recurrence chain.

All recurrence ops on DVE (Pool's Q7 engine disrupts SBUF arbitration and is
net-negative; measured repeatedly):
  m  = (u<=1)*u        STT (is_le, mult), ~1.1 ns/col
  u' = m*TAU + x'      STT (mult, add),   ~1.1 ns/col
Act engine: spike = sign(u-1) -> u8 (saturating f32->u8 convert gives exactly
(u>1): -1 -> 0, +1 -> 1); host decodes ==1. In/out DMAs on the sync queue.
"""

import numpy as np

import concourse.bacc as bacc
import concourse.mybir as mybir
from concourse.tile import TileContext
from concourse.bass_utils import run_bass_kernel_spmd

B, T, C, H, W = 32, 8, 128, 32, 32
HW = H * W
N_CORES = 8
B_LOC = B // N_CORES
F = B_LOC * HW  # 4096
TAU = 0.5
THRESH = 1.0

GW = [2048, 2048]  # column group widths
XBUFS = 7

_nc_cache = None


def build_nc():
    nc = bacc.Bacc("TRN2", target_bir_lowering=False)
    f32 = mybir.dt.float32
    u8 = mybir.dt.uint8
    op = mybir.AluOpType
    AF = mybir.ActivationFunctionType
    x = nc.dram_tensor("x", [C, T, F], f32, kind="ExternalInput")
    out = nc.dram_tensor("out", [C, T, F], u8, kind="ExternalOutput")

    assert sum(GW) == F
    offs = [sum(GW[:i]) for i in range(len(GW))]
    NG = len(GW)

    with TileContext(nc) as tc:
        import contextlib

        with contextlib.ExitStack() as ctx:
            cp = ctx.enter_context(tc.tile_pool(name="cp", bufs=1))
            neg1 = cp.tile([C, 1], f32, tag="neg1")
            nc.vector.memset(neg1[:], -1.0)

            xps = [
                ctx.enter_context(tc.tile_pool(name=f"x{g}", bufs=XBUFS))
                for g in range(NG)
            ]
            ups = [
                ctx.enter_context(tc.tile_pool(name=f"u{g}", bufs=3))
                for g in range(NG)
            ]
            mps = [
                ctx.enter_context(tc.tile_pool(name=f"m{g}", bufs=1))
                for g in range(NG)
            ]
            ops_ = [
                ctx.enter_context(tc.tile_pool(name=f"o{g}", bufs=3))
                for g in range(NG)
            ]

            xt = [[None] * NG for _ in range(T)]
            for t in range(T):
                for g in range(NG):
                    tile = xps[g].tile([C, GW[g]], f32, tag=f"x{g}", name=f"xt{g}")
                    nc.sync.dma_start(
                        out=tile[:], in_=x[:, t, offs[g] : offs[g] + GW[g]]
                    )
                    xt[t][g] = tile

            u = list(xt[0])
            for t in range(T):
                for g in range(NG):
                    parts = u[g] if isinstance(u[g], list) else [u[g]]
                    hw_ = GW[g] // len(parts)
                    for h, up_ in enumerate(parts):
                        ot = ops_[g].tile([C, hw_], u8, tag=f"o{g}", name=f"ot{g}")
                        nc.scalar.activation(
                            ot[:], up_[:], AF.Sign, bias=neg1[:], scale=1.0
                        )
                        nc.sync.dma_start(
                            out=out[:, t, offs[g] + h * hw_ : offs[g] + (h + 1) * hw_],
                            in_=ot[:],
                        )
                if t == T - 1:
                    break
                un = [None] * NG
                for g in range(NG):
                    m = mps[g].tile([C, GW[g]], f32, tag=f"m{g}", name=f"mt{g}")
                    nc.vector.scalar_tensor_tensor(
                        m[:], u[g][:], THRESH, u[g][:], op.is_le, op.mult
                    )
                    if t == T - 2:
                        # final link: update in half-tiles so the last acts
                        # and out-DMAs start earlier
                        hw_ = GW[g] // 2
                        halves = []
                        for h in range(2):
                            uh = ups[g].tile(
                                [C, hw_], f32, tag=f"u{g}", name=f"unh{g}"
                            )
                            nc.vector.scalar_tensor_tensor(
                                uh[:], m[:, h * hw_ : (h + 1) * hw_], TAU,
                                xt[t + 1][g][:, h * hw_ : (h + 1) * hw_],
                                op.mult, op.add,
                            )
                            halves.append(uh)
                        un[g] = halves
                    else:
                        un[g] = ups[g].tile(
                            [C, GW[g]], f32, tag=f"u{g}", name=f"un{g}"
                        )
                        nc.vector.scalar_tensor_tensor(
                            un[g][:], m[:], TAU, xt[t + 1][g][:], op.mult, op.add
                        )
                u = un
    nc.compile()
    return nc


def make_in_maps(x: np.ndarray) -> list[dict]:
    # x [B, T, C, H, W] -> per core [C, T, B_loc*HW]
    xs = np.ascontiguousarray(x).reshape(B, T, C, HW)
    return [
        {
            "x": np.ascontiguousarray(
                xs[i * B_LOC : (i + 1) * B_LOC].transpose(2, 1, 0, 3)
            ).reshape(C, T, F)
        }
        for i in range(N_CORES)
    ]


def kernel(x: np.ndarray) -> np.ndarray:
    global _nc_cache
    if _nc_cache is None:
        _nc_cache = build_nc()
    res = run_bass_kernel_spmd(_nc_cache, make_in_maps(x), list(range(N_CORES)))
    # out[c, t, b_loc*HW+hw]: spike iff value == 1 (sign in u8: -1 saturates to 0)
    parts = [
        (res.results[i]["out"].reshape(C, T, B_LOC, HW) == 1).transpose(2, 1, 0, 3)
        for i in range(N_CORES)
    ]
    full = np.concatenate(parts, axis=0)
    return full.reshape(B, T, C, H, W).astype(np.float32)
